# revision 44
# baseline (speedup 1.0000x reference)
"""Trainium2 Bass kernel: multi-head self-attention (B=2, L=2048, D=1024, H=16).

Sharding: 8 NeuronCores = 2 batches x 4 head-groups (4 heads per core).
Each core computes, for its batch and its 4 heads:
  qkv projection -> full attention -> partial out-projection (its heads'
  contribution to out @ w_out).  The host sums the 4 head-group partials per
  batch and adds b_out.

Device dataflow (all layouts chosen so that no on-chip transpose is needed):
  - host passes x^T  [D, L]  (d-major), so d is on SBUF partitions.
  - qkT  = w_qk.T @ x^T      -> [c=512, L]   (Q^T / K^T per head, dk on partitions)
  - V    = x^T.T  @ w_v_aug  -> [L, 260]     (k-major V, plus a ones column per
                                              head that yields the softmax
                                              denominator for free)
  - S^T  = (K^T)ᵀ@ Q^T       -> [k, q] tiles (per head; 2 heads packed in the
                                              128-partition dim, contraction 64)
  - P^T  = exp(S^T)          (no max-subtraction: |scores| <= ~10 in f32, safe)
  - O^T_aug = V_augᵀ... psum += V_aug[k,65].T-contract -> [65, q]
              rows 0-63 = unnormalized head output (dv-major), row 64 = sum_k P
  - normalize: O^T = O^T_aug[0:64] * bcast(1/row64)
  - y    = O^T_cat.T @ w_out_local -> [L, 1024] partial, DMA'd out.

All matmul operands are bf16 (full PE stream rate: 4-byte operands stream at
half rate); accumulation and the softmax math stay f32 in PSUM.  Measured on
HW: ~235 us per NEFF execution, rel err ~5e-3 vs the f32 reference.
"""

import sys

if "/opt/trn_rl_repo" not in sys.path:
    sys.path.insert(0, "/opt/trn_rl_repo")

import ml_dtypes
import numpy as np

import concourse.bass as bass
import concourse.tile as tile
from concourse import masks, mybir
from concourse.bass_utils import run_bass_kernel_spmd
from concourse.vector_clock import ScopedClock

B, L, D, H, DK = 2, 2048, 1024, 16, 64
HG = 4  # heads per core
F32 = mybir.dt.float32
F32R = mybir.dt.float32r
BF16 = mybir.dt.bfloat16
QC = 512  # l/q chunk width
NQ = L // QC  # 4 chunks
LT = L // 128  # 16 l tiles
KO = D // 128  # 8 contraction subtiles
CV = HG * (DK + 1)  # 260: v columns + per-head ones column

def _ensure_axon_hooks():
    """bass_utils imports antenv.axon_hooks when tracing is requested; the
    image's antenv lacks that module.  Register a null hook so a stray
    BASS_TRACE=1 degrades to an untraced run instead of an ImportError
    (test.py replaces this with the real ctypes hook for profiling)."""
    import sys as _sys

    if "antenv.axon_hooks" in _sys.modules:
        return
    try:
        import antenv
    except ImportError:
        return
    import types

    mod = types.ModuleType("antenv.axon_hooks")
    _state = {"h": None}
    mod.set_axon_ntff_profile_hook = lambda h: _state.__setitem__("h", h)
    mod.get_axon_ntff_profile_hook = lambda: _state["h"]
    _sys.modules["antenv.axon_hooks"] = mod
    antenv.axon_hooks = mod


_ensure_axon_hooks()

_PATCHED = False


def _patch_tile_drain():
    """This container's walrus rejects >1 sem wait on a ctrl instruction
    (setupSyncWait: 'Too many sync wait commands').  Tile's end-of-kernel
    drain accumulates one wait per outstanding semaphore; split the extras
    onto dedicated nops (same semantics: SP observes every sem before the
    final all-engine barrier)."""
    global _PATCHED
    if _PATCHED:
        return

    def _drain_and_barrier(self, tick_clock, wait_clock):
        nc = self.nc
        drain_inst = nc.sync.drain()
        wait_clock.add_sem_waits(
            drain_inst.ins, ScopedClock({None: tick_clock.global_clock})
        )
        si = drain_inst.ins.sync_info
        waits = list(si.on_wait or []) if si is not None else []
        if len(waits) > 1:
            si.on_wait = waits[:1]
            for w in waits[1:]:
                nop = nc.sync.nop()
                nsi = nop.ins.sync_info
                if nsi is None:
                    nop.ins.sync_info = mybir.SyncInfo(on_wait=[w], on_update=[])
                else:
                    nsi.on_wait = [w]
        nc.all_engine_barrier()
        popped = nc._tile_sem_poison_stack.pop()
        assert popped is self._sem_poison
        nc.clear_and_free_semaphores(list(self.sems.allocated().values()))
        nc.all_engine_barrier()

    tile.TileContext._drain_and_barrier = _drain_and_barrier
    _PATCHED = True


def _split_excess_waits(nc, max_waits=1):
    """This toolchain's walrus/ISA config allows only one sem wait per
    instruction, but Tile's wait assignment can attach several.  Hoist the
    extras onto same-engine nops immediately before the instruction (AND
    semantics preserved: the engine blocks on each in program order)."""
    for f in nc.m.functions:
        for blk in f.blocks:
            insts = list(blk.instructions)
            out = []
            changed = False
            for inst in insts:
                si = inst.sync_info
                waits = list(si.on_wait) if (si is not None and si.on_wait) else []
                if len(waits) > max_waits:
                    changed = True
                    for w in waits[:-max_waits]:
                        nop = mybir.InstNoOp(
                            name=f"I-wsplit-{nc.next_id()}",
                            engine=inst.engine,
                            ins=[],
                            outs=[],
                            sync_info=mybir.SyncInfo(on_wait=[w], on_update=[]),
                        )
                        nc.register_instruction(nop, overwrite=True)
                        out.append(nop)
                    si.on_wait = waits[-max_waits:]
                out.append(inst)
            if changed:
                blk.instructions = out

def _drop_redundant_ldweights(nc):
    """The walrus here serializes a ~97ns LDWEIGHTS before every matmul
    (ldw-opt is unavailable).  Where consecutive PE matmuls reuse the same
    stationary operand (stage B streams 4 chunks against one weight slice),
    the repeated loads are no-ops: drop any LDWEIGHTS whose operand AP matches
    the previous one with only MATMULs in between.  Skip any carrying sync."""
    for f in nc.m.functions:
        for blk in f.blocks:
            insts = list(blk.instructions)
            out = []
            prev_key = None
            changed = False
            for inst in insts:
                nm = type(inst).__name__
                if nm == "InstLdweights":
                    si = inst.sync_info
                    has_sync = si is not None and (si.on_wait or si.on_update)
                    key = (repr(inst.ins), getattr(inst, "tile_position", None))
                    if not has_sync and prev_key == key:
                        changed = True
                        continue  # weights already resident
                    prev_key = key if not has_sync else None
                elif nm == "InstMatmult":
                    pass  # matmuls don't disturb loaded weights
                elif inst.engine == mybir.EngineType.PE:
                    prev_key = None  # anything else on PE: be conservative
                out.append(inst)
            if changed:
                blk.instructions = out


def build_nc(zero_bias=True):
    _patch_tile_drain()
    nc = bass.Bass()
    xT = nc.declare_dram_parameter("xT", [D, L], BF16, isOutput=False)
    wqk = nc.declare_dram_parameter("wqk", [D, 512], BF16, isOutput=False)
    bqk = nc.declare_dram_parameter("bqk", [512], F32, isOutput=False)
    wv = nc.declare_dram_parameter("wv", [D, CV], BF16, isOutput=False)
    bv = nc.declare_dram_parameter("bv", [CV], BF16, isOutput=False)
    wout = nc.declare_dram_parameter("wout", [2 * 128, 1024], BF16, isOutput=False)
    ones = nc.declare_dram_parameter("ones", [1, L], BF16, isOutput=False)
    y = nc.declare_dram_parameter("out", [L, D], BF16, isOutput=True)

    Ident = mybir.ActivationFunctionType.Identity
    Exp = mybir.ActivationFunctionType.Exp

    with tile.TileContext(nc) as tc:
        with (
            tc.tile_pool(name="per", bufs=1) as per,
            tc.tile_pool(name="xtp", bufs=1) as xtp,
        ):
            wqk_ch = [
                per.tile([128, 512], BF16, tag=f"wqk{o}", name=f"wqk{o}")
                for o in range(KO)
            ]
            wv_sb = per.tile([128, KO, CV], BF16)
            wout_sb = per.tile([128, 2, 1024], BF16)
            bqk_sb = per.tile([128, 4], F32)
            bv_sb = per.tile([1, CV], BF16)
            ones_sb = per.tile([1, L], BF16)
            qkT_sb = per.tile([128, 4, L], BF16)
            v_sb = per.tile([128, LT, CV], BF16)
            oT_sb = per.tile([128, 2, L], BF16)
            xT_ch = [
                xtp.tile([128, L], BF16, tag=f"xt{o}", name=f"xt{o}")
                for o in range(KO)
            ]

            # loads in consumption order (wqk/xT chunk o feeds the o-th
            # accumulation step) so the projection matmuls start ~1us in and
            # the rest of the load streams under them.  4 queues; wqk split
            # into the slot-0/1 half (needed first) and the slot-2/3 half
            # (only needed mid-attention); wout last (needed at ~60us).
            qs = [nc.sync, nc.gpsimd, nc.scalar]
            _qi = [0]

            def q():
                e = qs[_qi[0] % 3]
                _qi[0] += 1
                return e

            scratch1 = per.tile([1, 1], F32)
            wtmp = per.tile([128, 512], BF16)
            # warm-up fodder: memset needs no DMA, so the PE can start ramping
            # its p-state immediately instead of idling until the first chunk
            nc.vector.memset(wtmp[:], 0.0)
            # dummy exp: pulls the ACT exp-table load (~2.7us) off the
            # critical path at the start of the attention phase
            nc.scalar.activation(scratch1[:], wtmp[0:1, 0:1], Exp)
            # Startup is HBM-BW-bound (~5.5MB before v_proj).  Round-robin
            # all three queues in strict consumption order, xT chunk ahead of
            # its wqk chunk; the 2-slot-interleaved projection consumes a
            # chunk pair every ~1.7us, which 3-queue aggregate BW sustains.
            for o in range(KO):
                q().dma_start(out=xT_ch[o][:], in_=xT[o * 128 : (o + 1) * 128, :])
                q().dma_start(out=wqk_ch[o][:], in_=wqk[o * 128 : (o + 1) * 128, :])
            nc.scalar.dma_start(out=bqk_sb[:], in_=bqk.rearrange("(s p) -> p s", p=128))
            for o in range(KO):
                q().dma_start(out=wv_sb[:, o, :], in_=wv[o * 128 : (o + 1) * 128, :])
            nc.gpsimd.dma_start(out=bv_sb[:], in_=bv[None, :])
            nc.sync.dma_start(out=ones_sb[:], in_=ones[:])
            nc.gpsimd.dma_start(out=wout_sb[:, 0, :], in_=wout[0:128, :])
            nc.sync.dma_start(out=wout_sb[:, 1, :], in_=wout[128:256, :])
            # f32 identity for the tail-normalize PE transposes (needed only
            # at ~195us; built on gpsimd after the DMA burst is queued)
            ident = per.tile([128, 128], F32)
            masks.make_identity(nc, ident[:])

            def v_proj(lt, pool):
                """V_aug k-tile lt = x @ w_v_aug.  With nonzero biases a K=1
                matmul adds b_v and the per-head ones column (the softmax
                rowsum); with zero biases the ones columns are written by a
                single strided memset instead (saves 16 matmuls)."""
                ps = pool.tile([128, CV], F32, tag="px", name="psv")
                for o in range(KO):
                    nc.tensor.matmul(
                        ps[:],
                        xT_ch[o][:, lt * 128 : (lt + 1) * 128],
                        wv_sb[:, o, :],
                        start=(o == 0),
                        stop=(zero_bias and o == KO - 1),
                    )
                if not zero_bias:
                    nc.tensor.matmul(
                        ps[:],
                        ones_sb[0:1, 0:128],
                        bv_sb[0:1, :],
                        start=False,
                        stop=True,
                    )
                nc.vector.tensor_copy(out=v_sb[:, lt, :], in_=ps[:])
                if zero_bias:
                    nc.vector.memset(v_sb[:, lt, 64 :: DK + 1], 1.0)

            def qk_proj_early_both(psB):
                """qkT slots 0 and 1 together, d-contraction as the outer
                loop: each x^T chunk feeds BOTH slots' matmuls back-to-back
                (1.7us of PE per chunk), matching the ~2us/chunk DMA arrival
                rate instead of starving on slot 0 then re-reading for slot 1.
                Uses all 8 PSUM banks.  Bias rides the DVE copyback (keeps
                ACT free for the attention exps)."""
                pss = [
                    [
                        psB.tile([128, QC], F32, tag=f"psqk{s}{n}", name=f"psqk{s}{n}")
                        for n in range(NQ)
                    ]
                    for s in range(2)
                ]
                # p-state warm-up: 8 throwaway matmuls on the memset tile keep
                # the PE streaming from t~0.5us while the first chunk lands
                pw = psB.tile([128, QC], F32, tag="psqk00", name="pwarm")
                for _ in range(8):
                    nc.tensor.matmul(
                        pw[:], wtmp[:, 0:128], wtmp[:], start=True, stop=True
                    )
                for o in range(KO):
                    for s in range(2):
                        for n in range(NQ):
                            nc.tensor.matmul(
                                pss[s][n][:],
                                wqk_ch[o][:, s * 128 : (s + 1) * 128],
                                xT_ch[o][:, n * QC : (n + 1) * QC],
                                start=(o == 0),
                                stop=(o == KO - 1),
                            )
                # copybacks split DVE/ACT: the psB pool-close barrier waits on
                # all 8, and a single engine takes ~6us serial.  ACT is idle
                # here (first exp is after the barrier anyway), and gpsimd
                # cannot read PSUM.
                for s in range(2):
                    for n in range(NQ):
                        if n % 2 == 0:
                            nc.vector.tensor_scalar(
                                out=qkT_sb[:, s, n * QC : (n + 1) * QC],
                                in0=pss[s][n][:],
                                scalar1=bqk_sb[:, s : s + 1],
                                scalar2=None,
                                op0=mybir.AluOpType.add,
                            )
                        else:
                            nc.scalar.activation(
                                qkT_sb[:, s, n * QC : (n + 1) * QC],
                                pss[s][n][:],
                                Ident,
                                bias=bqk_sb[:, s : s + 1],
                                scale=1.0,
                            )

            def qk_proj_late(s, npair, pool, state=None, o_range=None):
                """Two chunks of qkT slot s, emitted mid-attention (x^T fully
                resident).  o is the middle loop so the two chunks' matmuls
                share each weight slice back-to-back (redundant LDWEIGHTS get
                dropped); copyback on the vector engine (ACT is busy there).
                With state/o_range the o-loop can be emitted in slices so the
                PE burst spreads across several kt iterations."""
                ns = [2 * npair, 2 * npair + 1]
                if state is None:
                    pss = [
                        pool.tile([128, QC], F32, tag="px", name=f"psqkl{j}")
                        for j in range(2)
                    ]
                else:
                    pss = state
                for o in o_range if o_range is not None else range(KO):
                    for j in range(2):
                        nc.tensor.matmul(
                            pss[j][:],
                            wqk_ch[o][:, s * 128 : (s + 1) * 128],
                            xT_ch[o][:, ns[j] * QC : (ns[j] + 1) * QC],
                            start=(o == 0),
                            stop=(o == KO - 1),
                        )
                if o_range is not None and o_range[-1] != KO - 1:
                    return pss
                for j in range(2):
                    nc.vector.tensor_scalar(
                        out=qkT_sb[:, s, ns[j] * QC : (ns[j] + 1) * QC],
                        in0=pss[j][:],
                        scalar1=bqk_sb[:, s : s + 1],
                        scalar2=None,
                        op0=mybir.AluOpType.add,
                    )
                return pss

            with (
                tc.tile_pool(name="psB", bufs=1, space="PSUM") as psB,
            ):
                qk_proj_early_both(psB)
            # attention per head pair: even head on partitions 0-63, odd on
            # 64-127 (two row-group-packed K=64 matmuls run concurrently).
            # The second qkT projection (slots 2,3 for pair 1) is emitted
            # between pair 0 and pair 1 so the PE chews on it while ACT
            # finishes pair 0's exps.
            with (
                tc.tile_pool(name="pt", bufs=4) as ptp,
                tc.tile_pool(name="rcp", bufs=3) as rcp,
                tc.tile_pool(name="rdp", bufs=3, space="DRAM") as rdp,
                tc.tile_pool(name="psST", bufs=2, space="PSUM") as psST,
                tc.tile_pool(name="psOT", bufs=1, space="PSUM") as psOT,
                tc.tile_pool(name="psL", bufs=2, space="PSUM") as psL,
                tc.tile_pool(name="ysb", bufs=3) as ysb,
            ):

                def emit_out_part(qc, ltl, last=False):
                    """Out-projection for one 128-row l-tile of q chunk qc
                    (both pairs normalized).  Parts are spread through the
                    following q-chunk's kt loop (kt=3,7,11,15) so each ~850ns
                    PE burst lands in the exp-bound slack instead of stalling
                    the in-order PE behind the normalize chain.  Both n-halves
                    run as live PSUM groups in weights-major order so
                    consecutive matmuls share their stationary operand."""
                    lt = 4 * qc + ltl
                    pss = [
                        psL.tile([128, QC], F32, tag="px", name=f"psy{n2}")
                        for n2 in range(2)
                    ]
                    for s in range(2):
                        for n2 in range(2):
                            nc.tensor.matmul(
                                pss[n2][:],
                                oT_sb[:, s, lt * 128 : (lt + 1) * 128],
                                wout_sb[:, s, n2 * QC : (n2 + 1) * QC],
                                start=(s == 0),
                                stop=(s == 1),
                            )
                    for n2 in range(2):
                        yt = ysb.tile([128, QC], BF16, tag="yt")
                        if last and n2 == 1:
                            # final chunk: ACT is idle by now - split the
                            # copybacks across both engines
                            nc.scalar.copy(out=yt[:], in_=pss[n2][:])
                        else:
                            nc.vector.tensor_copy(out=yt[:], in_=pss[n2][:])
                        [nc.gpsimd, nc.sync][(2 * ltl + n2) % 2].dma_start(
                            out=y[lt * 128 : (lt + 1) * 128, n2 * QC : (n2 + 1) * QC],
                            in_=yt[:],
                        )

                def st_pair(sq, sk, qc, kt):
                    """S^T for both heads of the pair, written into the two
                    halves of one 2-bank PSUM tile so a single wide ACTIVATE
                    exps both (amortizes ACT's fixed per-op overhead)."""
                    ps2 = psST.tile([128, 2, QC], F32, tag="st2")
                    nc.tensor.matmul(
                        ps2[:, 0, :],
                        qkT_sb[0:64, sk, kt * 128 : (kt + 1) * 128],
                        qkT_sb[0:64, sq, qc * QC : (qc + 1) * QC],
                        start=True,
                        stop=True,
                    )
                    nc.tensor.matmul(
                        ps2[:, 1, :],
                        qkT_sb[64:128, sk, kt * 128 : (kt + 1) * 128],
                        qkT_sb[64:128, sq, qc * QC : (qc + 1) * QC],
                        start=True,
                        stop=True,
                    )
                    return ps2

                items = [(p2, qc) for p2 in range(2) for qc in range(NQ)]
                # V_aug for the first two k-tiles (from the attention psL
                # pool - a separate pool block would add a drain barrier);
                # the rest interleaves into pair 0's first q-chunk.
                for lt in range(2):
                    v_proj(lt, psL)
                # global software pipeline: the S^T pair runs one k-tile ahead
                # of its exp/O^T consumers, ACROSS (pair, q-chunk) boundaries,
                # so the PE/ACT streams never drain at a transition
                sts = st_pair(0, 1, 0, 0)
                ql_state = None
                for idx, (p2, qc) in enumerate(items):
                    sq = 2 * p2  # q slot in qkT_sb
                    sk = 2 * p2 + 1  # k slot
                    if True:
                        if p2 == 0 and qc > 0:
                            # second qkT projection, spread so each item's
                            # extra PE work roughly matches its ACT slack
                            # ((3,1) lands in pair1-qc0, the only chunk with
                            # no other PE filler; it is consumed from kt=8)
                            for s, npair in [
                                [(2, 0)],
                                [(2, 1)],
                                [(3, 0)],
                            ][qc - 1]:
                                qk_proj_late(s, npair, psL)
                        po_e = psOT.tile([65, QC], F32, tag="ote")
                        po_o = psOT.tile([65, QC], F32, tag="oto")
                        for kt in range(LT):
                            ps2 = sts
                            pt2 = ptp.tile([128, 2, QC], BF16, tag="pt2")
                            nc.scalar.activation(pt2[:], ps2[:], Exp)
                            if kt + 1 < LT:
                                sts = st_pair(sq, sk, qc, kt + 1)
                            elif idx + 1 < len(items):
                                np2, nqc = items[idx + 1]
                                sts = st_pair(2 * np2, 2 * np2 + 1, nqc, 0)
                            if p2 == 0 and qc == 0 and kt < LT - 2:
                                # stream the rest of the V projection one
                                # k-tile ahead of its O^T consumer
                                v_proj(kt + 2, psL)
                            if p2 == 1 and qc == 0 and kt in (1, 3, 5):
                                # (3,1) split into o-slices so each ~1us burst
                                # lands in per-kt exp slack; must complete by
                                # kt=5 (kt=7 issues st_pair(8) which consumes
                                # it - emitting later would deadlock the
                                # in-order PE behind its own dependency)
                                ql_state = qk_proj_late(
                                    3,
                                    1,
                                    psL,
                                    state=ql_state,
                                    o_range={1: range(0, 3), 3: range(3, 6), 5: range(6, 8)}[kt],
                                )
                            if p2 == 1 and qc > 0 and kt in (10, 12, 14, 15):
                                emit_out_part(qc - 1, {10: 0, 12: 1, 14: 2, 15: 3}[kt])
                            nc.tensor.matmul(
                                po_e[:],
                                v_sb[:, kt, (2 * p2) * 65 : (2 * p2) * 65 + 65],
                                pt2[:, 0, :],
                                start=(kt == 0),
                                stop=(kt == LT - 1),
                            )
                            nc.tensor.matmul(
                                po_o[:],
                                v_sb[:, kt, (2 * p2 + 1) * 65 : (2 * p2 + 1) * 65 + 65],
                                pt2[:, 1, :],
                                start=(kt == 0),
                                stop=(kt == LT - 1),
                            )
                        last_item = idx == len(items) - 1
                        for he, po in ((0, po_e), (1, po_o)):
                            oT_dst = oT_sb[
                                he * 64 : (he + 1) * 64, p2, qc * QC : (qc + 1) * QC
                            ]
                            po_sb = rcp.tile([65, QC], F32, tag="po_sb")
                            if not last_item:
                                # Stage O^T_aug to SBUF (frees the PSUM bank);
                                # the rowsum bounces through DRAM into [128,4]
                                # so the DVE reciprocal has a tiny free dim
                                # (reciprocal costs ~6.5ns per free-element),
                                # then back broadcast across 64 partitions.
                                # Each DMA hop has ~3us ring latency; the
                                # rowsum row is copied first (separately) so
                                # the chain's first DMA launches ~0.6us sooner.
                                rsum = rcp.tile([1, QC], F32, tag="rsum")
                                nc.vector.tensor_copy(out=rsum[:], in_=po[64:65, :])
                                nc.vector.tensor_copy(
                                    out=po_sb[0:64, :], in_=po[0:64, :]
                                )
                                rb = rcp.tile([64, QC], F32, tag="rb")
                                rp = rcp.tile([128, QC // 128], F32, tag="rp")
                                rd = rdp.tile([1, QC], F32, tag="rd")
                                rd2 = rdp.tile([1, QC], F32, tag="rd2")
                                nc.sync.dma_start(out=rd[:], in_=rsum[:])
                                nc.sync.dma_start(
                                    out=rp[:],
                                    in_=rd[0, :].rearrange("(p f) -> p f", p=128),
                                )
                                nc.vector.reciprocal(out=rp[:], in_=rp[:])
                                nc.sync.dma_start(
                                    out=rd2[0, :].rearrange("(p f) -> p f", p=128),
                                    in_=rp[:],
                                )
                                nc.sync.dma_start(
                                    out=rb[:], in_=rd2[0:1, :].to_broadcast([64, QC])
                                )
                                nc.vector.tensor_mul(
                                    out=oT_dst, in0=po_sb[0:64, :], in1=rb[:]
                                )
                            else:
                                # Final q-chunk: the chain is fully exposed, so
                                # route the cross-partition moves through the
                                # (idle) PE instead of ~3us/hop DMA bounces:
                                # transpose rowsum to [128,4], fast reciprocal,
                                # transpose back, outer-product broadcast.
                                rsum = rcp.tile([1, QC], F32, tag="rsum")
                                nc.vector.tensor_copy(out=rsum[:], in_=po[64:65, :])
                                nc.vector.tensor_copy(
                                    out=po_sb[0:64, :], in_=po[0:64, :]
                                )
                                rp_ps = psL.tile([128, QC], F32, tag="px", name="rpps")
                                for k in range(4):
                                    nc.tensor.transpose(
                                        rp_ps[:, k : k + 1],
                                        rsum[0:1, 128 * k : 128 * (k + 1)],
                                        ident[0:1, 0:1],
                                    )
                                rp = rcp.tile([128, QC // 128], F32, tag="rp")
                                nc.vector.reciprocal(out=rp[:], in_=rp_ps[:, 0:4])
                                rr_ps = psL.tile([128, QC], F32, tag="px", name="rrps")
                                for k in range(4):
                                    nc.tensor.transpose(
                                        rr_ps[0:1, 128 * k : 128 * (k + 1)],
                                        rp[:, k : k + 1],
                                        ident[:, 0:128],
                                    )
                                rrow = rcp.tile([1, QC], BF16, tag="rrbf")
                                nc.vector.tensor_copy(out=rrow[:], in_=rr_ps[0:1, :])
                                rb_ps = psL.tile([128, QC], F32, tag="px", name="rbps")
                                nc.tensor.matmul(
                                    rb_ps[0:64, :],
                                    ones_sb[0:1, 0:64],
                                    rrow[0:1, :],
                                    start=True,
                                    stop=True,
                                )
                                nc.vector.tensor_mul(
                                    out=oT_dst, in0=po_sb[0:64, :], in1=rb_ps[0:64, :]
                                )

                for ltl in range(NQ):
                    emit_out_part(NQ - 1, ltl, last=(ltl == NQ - 1))
    _drop_redundant_ldweights(nc)
    _split_excess_waits(nc)
    return nc


def make_in_maps(x, w_qkv, b_qkv, w_out):
    """Per-core input shards.  Core i: batch i//4, head group i%4 (4 heads).

    w_qk column order per core: slots of 128 = (pair0 q | pair0 k | pair1 q |
    pair1 k), each slot = [even head (64) | odd head (64)].  The 1/sqrt(dk)
    scale is folded into the q columns (and q bias entries).
    """
    in_maps = []
    for core in range(8):
        b, g = divmod(core, 4)
        heads = [4 * g + j for j in range(HG)]
        xT = np.ascontiguousarray(x[b].T)
        cols, bias = [], []
        for pair in range(2):
            for qk in range(2):
                for j in range(2):
                    h = heads[2 * pair + j]
                    base = h * 3 * DK + qk * DK
                    c = w_qkv[:, base : base + DK]
                    bb = b_qkv[base : base + DK]
                    if qk == 0:
                        c = c * (1.0 / np.sqrt(DK))
                        bb = bb * (1.0 / np.sqrt(DK))
                    cols.append(c)
                    bias.append(bb)
        wqk = np.ascontiguousarray(np.concatenate(cols, axis=1), dtype=np.float32)
        bqk = np.concatenate(bias).astype(np.float32)
        wv = np.zeros((D, CV), np.float32)
        bv = np.zeros((CV,), np.float32)
        for j, h in enumerate(heads):
            base = h * 3 * DK + 2 * DK
            wv[:, 65 * j : 65 * j + 64] = w_qkv[:, base : base + DK]
            bv[65 * j : 65 * j + 64] = b_qkv[base : base + DK]
            bv[65 * j + 64] = 1.0
        wo = np.ascontiguousarray(w_out[g * 256 : (g + 1) * 256, :], dtype=np.float32)
        bf = ml_dtypes.bfloat16
        in_maps.append(
            {
                "xT": xT.astype(bf),
                "wqk": wqk.astype(bf),
                "bqk": bqk,
                "wv": wv.astype(bf),
                "bv": bv.astype(bf),
                "wout": wo.astype(bf),
                "ones": np.ones((1, L), bf),
            }
        )
    return in_maps


def kernel(**inputs):
    x = np.asarray(inputs["x"], np.float32)
    w_qkv = np.asarray(inputs["w_qkv"], np.float32)
    b_qkv = np.asarray(inputs["b_qkv"], np.float32)
    w_out = np.asarray(inputs["w_out"], np.float32)
    b_out = np.asarray(inputs["b_out"], np.float32)

    in_maps = make_in_maps(x, w_qkv, b_qkv, w_out)
    nc = build_nc(zero_bias=not bool(np.any(b_qkv)))
    res = run_bass_kernel_spmd(nc, in_maps, core_ids=list(range(8)))
    kernel.last_results = res

    out = np.zeros((B, L, D), np.float32)
    for core in range(8):
        out[core // 4] += res.results[core]["out"].astype(np.float32)
    out += b_out[None, None, :]
    return out


kernel.last_results = None

